# revision 5
# baseline (speedup 1.0000x reference)
"""Distributed attention layer for 8 TRN2 NeuronCores.

Problem: y = softmax((x Wq^T)(K Wk^T)^T / sqrt(D)) (V Wv^T)
with B=4, S=T=4096, D=256, fp32.

Sharding: 8 shards = 4 batches x 2 query-row halves. Each core gets its
2048 query rows, the full K/V of its batch (4096x256), and replicated
weights; no collectives are needed.

Per-core dataflow (all heavy matmuls in float32r: ~4x faster than fp32
on the PE at ~1.6e-4 rel err):
  1. PE-transpose 128-row tiles of raw Q/K, project with transposed
     weights -> qT [e, s], kT [e, t] (feature dim on partitions).
  2. Load V, append a ones column, round to f32r -> v_aug [t, 257].
  3. scoresT[t_tile, s_blk] = kT_chunk^T @ qT  (accumulate e-chunks in
     PSUM), then exp(scale * scores) on the scalar engine -> expT tiles.
  4. z_aug[s_tile, :] = sum_t expT^T @ v_aug; column 256 is the softmax
     denominator (exp sums), so normalization is one reciprocal + one
     per-partition scalar multiply (no max subtraction needed: scores
     are ~N(0,1), exp never overflows fp32).
  5. PE-transpose z, project with WvT -> y tile, DMA out.
"""

import sys

for _p in ("/opt/trn_rl_repo", "/root/.axon_site/_ro/trn_rl_repo"):
    if _p not in sys.path:
        sys.path.append(_p)

import numpy as np

B, S_FULL, T_FULL, D_MODEL = 4, 4096, 4096, 256
N_CORES = 8

_compiled_cache = {}


def build_attention_nc(S=2048, T=4096, D=256, SB=512, TG=512):
    """Build the per-core Bass program.

    S: query rows per core; T: kv rows; D: model dim (must be 256);
    SB: s-block width for the scores/exp stage; TG: token group width
    for the transpose+project prep stage.
    """
    import concourse.bass as bass  # noqa: F401  (engine types resolve via nc)
    import concourse.mybir as mybir
    import concourse.tile as tile
    from concourse import bacc
    from concourse.masks import make_identity

    F32 = mybir.dt.float32
    F32R = mybir.dt.float32r
    AF = mybir.ActivationFunctionType

    assert D == 256
    nD = D // 128           # feature chunks
    nT = T // 128           # kv tiles
    nSB = S // SB           # s blocks
    nTG = T // TG
    nSG = S // TG
    scale = 1.0 / float(np.sqrt(D))

    nc = bacc.Bacc("TRN2", target_bir_lowering=False, debug=False,
                   num_devices=N_CORES)
    q_in = nc.declare_dram_parameter("query", [S, D], F32, isOutput=False)
    k_in = nc.declare_dram_parameter("key", [T, D], F32, isOutput=False)
    v_in = nc.declare_dram_parameter("value", [T, D], F32, isOutput=False)
    w_in = {
        "q": nc.declare_dram_parameter("Wq", [D, D], F32, isOutput=False),
        "k": nc.declare_dram_parameter("Wk", [D, D], F32, isOutput=False),
        "v": nc.declare_dram_parameter("Wv", [D, D], F32, isOutput=False),
    }
    out = nc.declare_dram_parameter("out", [S, D], F32, isOutput=True)

    with tile.TileContext(nc) as tc:
        with (
            tc.tile_pool(name="persist", bufs=1) as persist,
            tc.tile_pool(name="stage", bufs=4) as stage,
            tc.tile_pool(name="xt", bufs=2) as xtp,
            tc.tile_pool(name="work", bufs=3) as work,
            tc.tile_pool(name="exps", bufs=1) as exps,
            tc.tile_pool(name="psA", bufs=2, space="PSUM") as psA,   # transposes
            tc.tile_pool(name="psB", bufs=2, space="PSUM") as psB,   # projections
            tc.tile_pool(name="psS", bufs=2, space="PSUM") as psS,   # scores
            tc.tile_pool(name="psY", bufs=2, space="PSUM") as psY,   # attn@V
        ):
            ident = persist.tile([128, 128], F32, tag="ident", name="ident")
            make_identity(nc, ident[:])

            # ---- weights: W[e, d] -> WT_r[dc] = [128 d, D e] (f32r) ----
            wT = {}
            for wname, wdram in w_in.items():
                wT[wname] = [persist.tile([128, D], F32R, tag=f"w_{wname}{dc}", name=f"w_{wname}{dc}")
                             for dc in range(nD)]
                for ec in range(nD):
                    wst = stage.tile([128, D], F32, tag="wstage", name="wstage")
                    nc.sync.dma_start(wst[:], wdram[ec * 128:(ec + 1) * 128, :])
                    for dc in range(nD):
                        pt = psA.tile([128, 128], F32, tag="tr", name="tr_ps")
                        nc.tensor.transpose(
                            pt[:], wst[:, dc * 128:(dc + 1) * 128], ident[:])
                        nc.vector.tensor_copy(
                            wT[wname][dc][:, ec * 128:(ec + 1) * 128], pt[:])

            # ---- V: load, append ones columns (2: f32r needs even free
            # dim), round to f32r ----
            v_r = [persist.tile([128, D + 2], F32R, tag=f"v{tc_i}", name=f"v{tc_i}")
                   for tc_i in range(nT)]
            for tc_i in range(nT):
                vst = stage.tile([128, D + 2], F32, tag="vstage", name="vstage")
                nc.gpsimd.memset(vst[:, D:D + 2], 1.0)
                nc.sync.dma_start(vst[:, 0:D], v_in[tc_i * 128:(tc_i + 1) * 128, :])
                nc.gpsimd.tensor_copy(v_r[tc_i][:], vst[:])

            # ---- transpose + project K -> kT_r, Q -> qT_r ----
            kT_r = [persist.tile([128, T], F32R, tag=f"kT{ec}", name=f"kT{ec}")
                    for ec in range(nD)]
            qT_r = [persist.tile([128, S], F32R, tag=f"qT{ec}", name=f"qT{ec}")
                    for ec in range(nD)]

            def prep(x_dram, n_groups, wTx, dstT):
                for g in range(n_groups):
                    xT = [xtp.tile([128, TG], F32R, tag=f"xt{dc}", name=f"xt{dc}")
                          for dc in range(nD)]
                    for j in range(TG // 128):
                        xst = stage.tile([128, D], F32, tag="xstage", name="xstage")
                        row0 = g * TG + j * 128
                        nc.sync.dma_start(xst[:], x_dram[row0:row0 + 128, :])
                        for dc in range(nD):
                            pt = psA.tile([128, 128], F32, tag="tr", name="tr_ps")
                            nc.tensor.transpose(
                                pt[:], xst[:, dc * 128:(dc + 1) * 128], ident[:])
                            nc.vector.tensor_copy(
                                xT[dc][:, j * 128:(j + 1) * 128], pt[:])
                    for ec in range(nD):
                        pp = psB.tile([128, TG], F32, tag="proj", name="proj_ps")
                        for dc in range(nD):
                            nc.tensor.matmul(
                                pp[:], wTx[dc][:, ec * 128:(ec + 1) * 128],
                                xT[dc][:], start=(dc == 0), stop=(dc == nD - 1))
                        nc.vector.tensor_copy(
                            dstT[ec][:, g * TG:(g + 1) * TG], pp[:])

            prep(k_in, nTG, wT["k"], kT_r)
            prep(q_in, nSG, wT["q"], qT_r)

            # ---- main loop over s blocks ----
            for b in range(nSB):
                s0 = b * SB
                expT = [exps.tile([128, SB], F32R, tag=f"e{tc_i}", name=f"e{tc_i}")
                        for tc_i in range(nT)]
                for tc_i in range(nT):
                    ps = psS.tile([128, SB], F32, tag="sc", name="sc_ps")
                    for ec in range(nD):
                        nc.tensor.matmul(
                            ps[:], kT_r[ec][:, tc_i * 128:(tc_i + 1) * 128],
                            qT_r[ec][:, s0:s0 + SB],
                            start=(ec == 0), stop=(ec == nD - 1))
                    nc.scalar.activation(expT[tc_i][:], ps[:], AF.Exp,
                                         scale=scale)
                for ss in range(SB // 128):
                    st = (s0 + ss * 128)
                    yp = psY.tile([128, D + 2], F32, tag="yp", name="yp_ps")
                    for tc_i in range(nT):
                        nc.tensor.matmul(
                            yp[:], expT[tc_i][:, ss * 128:(ss + 1) * 128],
                            v_r[tc_i][:],
                            start=(tc_i == 0), stop=(tc_i == nT - 1))
                    rec = work.tile([128, 1], F32, tag="rec", name="rec")
                    nc.vector.reciprocal(rec[:], yp[:, D:D + 1])
                    zn = work.tile([128, D], F32, tag="zn", name="zn")
                    nc.vector.tensor_scalar_mul(zn[:], yp[:, 0:D], rec[:])
                    zt = []
                    for dc in range(nD):
                        pt = psA.tile([128, 128], F32, tag="tr", name="tr_ps")
                        nc.tensor.transpose(
                            pt[:], zn[:, dc * 128:(dc + 1) * 128], ident[:])
                        z1 = work.tile([128, 128], F32R, tag=f"zt{dc}", name=f"zt{dc}")
                        nc.vector.tensor_copy(z1[:], pt[:])
                        zt.append(z1)
                    yp2 = psB.tile([128, D], F32, tag="proj", name="yp2_ps")
                    for dc in range(nD):
                        nc.tensor.matmul(yp2[:], zt[dc][:], wT["v"][dc][:, 0:D],
                                         start=(dc == 0), stop=(dc == nD - 1))
                    yo = work.tile([128, D], F32, tag="yo", name="yo")
                    nc.vector.tensor_copy(yo[:], yp2[:])
                    nc.sync.dma_start(out[st:st + 128, :], yo[:])

    nc.compile()
    return nc


def _get_compiled(key, **kwargs):
    if key not in _compiled_cache:
        _compiled_cache[key] = build_attention_nc(**kwargs)
    return _compiled_cache[key]


def kernel(query, key, value, Wq, Wk, Wv):
    from concourse.bass_utils import run_bass_kernel_spmd

    query = np.ascontiguousarray(query, dtype=np.float32)
    key = np.ascontiguousarray(key, dtype=np.float32)
    value = np.ascontiguousarray(value, dtype=np.float32)
    Wq = np.ascontiguousarray(Wq, dtype=np.float32)
    Wk = np.ascontiguousarray(Wk, dtype=np.float32)
    Wv = np.ascontiguousarray(Wv, dtype=np.float32)

    S_shard = S_FULL // 2
    nc = _get_compiled("full", S=S_shard, T=T_FULL, D=D_MODEL)

    in_maps = []
    for i in range(N_CORES):
        b, h = i // 2, i % 2
        in_maps.append({
            "query": query[b, h * S_shard:(h + 1) * S_shard, :],
            "key": key[b],
            "value": value[b],
            "Wq": Wq, "Wk": Wk, "Wv": Wv,
        })
    res = run_bass_kernel_spmd(nc, in_maps, core_ids=list(range(N_CORES)))
    y = np.empty((B, S_FULL, D_MODEL), dtype=np.float32)
    for i in range(N_CORES):
        b, h = i // 2, i % 2
        y[b, h * S_shard:(h + 1) * S_shard, :] = res.results[i]["out"]
    return y


# revision 6
# speedup vs baseline: 1.2221x; 1.2221x over previous
"""Distributed attention layer for 8 TRN2 NeuronCores.

Problem: y = softmax((x Wq^T)(K Wk^T)^T / sqrt(D)) (V Wv^T)
with B=4, S=T=4096, D=256, fp32.

Sharding: 8 shards = 4 batches x 2 query-row halves. Each core gets its
2048 query rows, the full K/V of its batch (4096x256), and replicated
weights; no collectives are needed.

Per-core dataflow (matmul operands in bf16: fp32 accumulate in PSUM,
fast weight loads; fp32-path data stays fp32 until the matmul-facing
copy):
  1. PE-transpose 128-row tiles of raw Q/K, project with transposed
     weights -> qT [e, s], kT [e, t] (feature dim on partitions),
     per 512-token group so dependent matmuls unblock early.
  2. Load V, append ones columns, round to bf16 -> v_aug [t, 258].
  3. scoresT[t_tile, s_blk] = kT_chunk^T @ qT (accumulate e-chunks in
     PSUM), then exp(scale * scores) on the scalar engine -> expT tiles.
     Scores for s-block 0 are interleaved with the K prep groups so the
     tensor engine warms up (HAM) early and prep overlaps the main loop.
  4. z_aug[s_tile, :] = sum_t expT^T @ v_aug; column 256 is the softmax
     denominator (exp sums), so normalization is one reciprocal + one
     per-partition scalar multiply (no max subtraction needed: scores
     are ~N(0,1), exp never overflows fp32).
  5. PE-transpose z, project with WvT -> y tile, DMA out.
"""

import sys

for _p in ("/opt/trn_rl_repo", "/root/.axon_site/_ro/trn_rl_repo"):
    if _p not in sys.path:
        sys.path.append(_p)

import numpy as np

B, S_FULL, T_FULL, D_MODEL = 4, 4096, 4096, 256
N_CORES = 8

_compiled_cache = {}


def build_attention_nc(S=2048, T=4096, D=256, SB=512, TG=512):
    """Build the per-core Bass program.

    S: query rows per core; T: kv rows; D: model dim (must be 256);
    SB: s-block width for the scores/exp stage (must equal TG);
    TG: token group width for the transpose+project prep stage.
    """
    import concourse.bass as bass  # noqa: F401
    import concourse.mybir as mybir
    import concourse.tile as tile
    from concourse import bacc
    from concourse.masks import make_identity

    F32 = mybir.dt.float32
    BF16 = mybir.dt.bfloat16
    AF = mybir.ActivationFunctionType

    assert D == 256
    assert SB == TG
    nD = D // 128           # feature chunks
    nT = T // 128           # kv tiles
    nSB = S // SB           # s blocks
    nTG = T // TG           # kv groups
    nSG = S // TG           # q groups
    tpg = TG // 128         # tiles per group
    scale = 1.0 / float(np.sqrt(D))

    nc = bacc.Bacc("TRN2", target_bir_lowering=False, debug=False,
                   num_devices=N_CORES)
    q_in = nc.declare_dram_parameter("query", [S, D], F32, isOutput=False)
    k_in = nc.declare_dram_parameter("key", [T, D], F32, isOutput=False)
    v_in = nc.declare_dram_parameter("value", [T, D], F32, isOutput=False)
    w_in = {
        "q": nc.declare_dram_parameter("Wq", [D, D], F32, isOutput=False),
        "k": nc.declare_dram_parameter("Wk", [D, D], F32, isOutput=False),
        "v": nc.declare_dram_parameter("Wv", [D, D], F32, isOutput=False),
    }
    out = nc.declare_dram_parameter("out", [S, D], F32, isOutput=True)

    with tile.TileContext(nc) as tc:
        with (
            tc.tile_pool(name="persist", bufs=1) as persist,
            tc.tile_pool(name="stage", bufs=4) as stage,
            tc.tile_pool(name="xt", bufs=2) as xtp,
            tc.tile_pool(name="work", bufs=3) as work,
            tc.tile_pool(name="exps", bufs=2) as exps,
            tc.tile_pool(name="psA", bufs=2, space="PSUM") as psA,   # transposes
            tc.tile_pool(name="psB", bufs=2, space="PSUM") as psB,   # projections
            tc.tile_pool(name="psS", bufs=2, space="PSUM") as psS,   # scores
            tc.tile_pool(name="psY", bufs=2, space="PSUM") as psY,   # attn@V
        ):
            ident = persist.tile([128, 128], F32, tag="ident", name="ident")
            make_identity(nc, ident[:])

            # ---- weights: W[e, d] -> WT[dc] = [128 d, D e] (bf16) ----
            wT = {}
            for wname, wdram in w_in.items():
                wT[wname] = [persist.tile([128, D], BF16,
                                          tag=f"w_{wname}{dc}",
                                          name=f"w_{wname}{dc}")
                             for dc in range(nD)]
                for ec in range(nD):
                    wst = stage.tile([128, D], F32, tag="wstage", name="wstage")
                    nc.sync.dma_start(wst[:], wdram[ec * 128:(ec + 1) * 128, :])
                    for dc in range(nD):
                        pt = psA.tile([128, 128], F32, tag="tr", name="tr_ps")
                        nc.tensor.transpose(
                            pt[:], wst[:, dc * 128:(dc + 1) * 128], ident[:])
                        nc.vector.tensor_copy(
                            wT[wname][dc][:, ec * 128:(ec + 1) * 128], pt[:])

            # ---- V: load, append ones columns, round to bf16 ----
            v_r = [persist.tile([128, D + 2], BF16, tag=f"v{i}", name=f"v{i}")
                   for i in range(nT)]
            for i in range(nT):
                vst = stage.tile([128, D + 2], F32, tag="vstage", name="vstage")
                nc.gpsimd.memset(vst[:, D:D + 2], 1.0)
                nc.sync.dma_start(vst[:, 0:D], v_in[i * 128:(i + 1) * 128, :])
                nc.gpsimd.tensor_copy(v_r[i][:], vst[:])

            # ---- per-group transposed projections ----
            # kT[g][ec]: [128 e, TG t] for kv group g; qT[g][ec] likewise.
            kT = [[persist.tile([128, TG], BF16, tag=f"kT{g}_{ec}",
                                name=f"kT{g}_{ec}") for ec in range(nD)]
                  for g in range(nTG)]
            qT = [[persist.tile([128, TG], BF16, tag=f"qT{g}_{ec}",
                                name=f"qT{g}_{ec}") for ec in range(nD)]
                  for g in range(nSG)]

            def prep_group(x_dram, g, wTx, dstT):
                """Transpose 128-row tiles of group g and project."""
                xT = [xtp.tile([128, TG], BF16, tag=f"xt{dc}", name=f"xt{dc}")
                      for dc in range(nD)]
                for j in range(tpg):
                    xst = stage.tile([128, D], F32, tag="xstage", name="xstage")
                    row0 = g * TG + j * 128
                    nc.sync.dma_start(xst[:], x_dram[row0:row0 + 128, :])
                    for dc in range(nD):
                        pt = psA.tile([128, 128], F32, tag="tr", name="tr_ps")
                        nc.tensor.transpose(
                            pt[:], xst[:, dc * 128:(dc + 1) * 128], ident[:])
                        nc.vector.tensor_copy(
                            xT[dc][:, j * 128:(j + 1) * 128], pt[:])
                for ec in range(nD):
                    pp = psB.tile([128, TG], F32, tag="proj", name="proj_ps")
                    for dc in range(nD):
                        nc.tensor.matmul(
                            pp[:], wTx[dc][:, ec * 128:(ec + 1) * 128],
                            xT[dc][:], start=(dc == 0), stop=(dc == nD - 1))
                    nc.vector.tensor_copy(dstT[ec][:], pp[:])

            def scores_chunk(b, expT, tc_lo, tc_hi):
                """scoresT + exp for t-tiles [tc_lo, tc_hi) of s-block b."""
                s0 = b * SB
                for tc_i in range(tc_lo, tc_hi):
                    g, r = tc_i // tpg, tc_i % tpg
                    ps = psS.tile([128, SB], F32, tag="sc", name="sc_ps")
                    for ec in range(nD):
                        nc.tensor.matmul(
                            ps[:], kT[g][ec][:, r * 128:(r + 1) * 128],
                            qT[b][ec][:],
                            start=(ec == 0), stop=(ec == nD - 1))
                    nc.scalar.activation(expT[tc_i][:], ps[:], AF.Exp,
                                         scale=scale)

            def attn_v_block(b, expT):
                """attn @ [V|1], normalize, transpose, Wv-project, store."""
                s0 = b * SB
                for ss in range(SB // 128):
                    st = s0 + ss * 128
                    yp = psY.tile([128, D + 2], F32, tag="yp", name="yp_ps")
                    for tc_i in range(nT):
                        nc.tensor.matmul(
                            yp[:], expT[tc_i][:, ss * 128:(ss + 1) * 128],
                            v_r[tc_i][:],
                            start=(tc_i == 0), stop=(tc_i == nT - 1))
                    rec = work.tile([128, 1], F32, tag="rec", name="rec")
                    nc.vector.reciprocal(rec[:], yp[:, D:D + 1])
                    zn = work.tile([128, D], F32, tag="zn", name="zn")
                    nc.vector.tensor_scalar_mul(zn[:], yp[:, 0:D], rec[:])
                    zt = []
                    for dc in range(nD):
                        pt = psA.tile([128, 128], F32, tag="tr", name="tr_ps")
                        nc.tensor.transpose(
                            pt[:], zn[:, dc * 128:(dc + 1) * 128], ident[:])
                        z1 = work.tile([128, 128], BF16, tag=f"zt{dc}",
                                       name=f"zt{dc}")
                        nc.vector.tensor_copy(z1[:], pt[:])
                        zt.append(z1)
                    yp2 = psB.tile([128, D], F32, tag="proj", name="yp2_ps")
                    for dc in range(nD):
                        nc.tensor.matmul(yp2[:], zt[dc][:],
                                         wT["v"][dc][:, 0:D],
                                         start=(dc == 0), stop=(dc == nD - 1))
                    yo = work.tile([128, D], F32, tag="yo", name="yo")
                    nc.vector.tensor_copy(yo[:], yp2[:])
                    nc.sync.dma_start(out[st:st + 128, :], yo[:])

            # ---- emission: interleave block-0 scores with K prep so the
            # tensor engine gets dense matmul work (HAM warm) early ----
            prep_group(q_in, 0, wT["q"], qT[0])
            expT0 = [exps.tile([128, SB], BF16, tag=f"e{i}", name=f"e{i}")
                     for i in range(nT)]
            chunks_per_group = nT // nTG  # t-tiles unlocked per K group
            for g in range(nTG):
                prep_group(k_in, g, wT["k"], kT[g])
                scores_chunk(0, expT0, g * chunks_per_group,
                             (g + 1) * chunks_per_group)
            for g in range(1, nSG):
                prep_group(q_in, g, wT["q"], qT[g])
            attn_v_block(0, expT0)

            for b in range(1, nSB):
                expT = [exps.tile([128, SB], BF16, tag=f"e{i}", name=f"e{i}")
                        for i in range(nT)]
                scores_chunk(b, expT, 0, nT)
                attn_v_block(b, expT)

    nc.compile()
    return nc


def _get_compiled(key, **kwargs):
    if key not in _compiled_cache:
        _compiled_cache[key] = build_attention_nc(**kwargs)
    return _compiled_cache[key]


def kernel(query, key, value, Wq, Wk, Wv):
    from concourse.bass_utils import run_bass_kernel_spmd

    query = np.ascontiguousarray(query, dtype=np.float32)
    key = np.ascontiguousarray(key, dtype=np.float32)
    value = np.ascontiguousarray(value, dtype=np.float32)
    Wq = np.ascontiguousarray(Wq, dtype=np.float32)
    Wk = np.ascontiguousarray(Wk, dtype=np.float32)
    Wv = np.ascontiguousarray(Wv, dtype=np.float32)

    S_shard = S_FULL // 2
    nc = _get_compiled("full", S=S_shard, T=T_FULL, D=D_MODEL)

    in_maps = []
    for i in range(N_CORES):
        b, h = i // 2, i % 2
        in_maps.append({
            "query": query[b, h * S_shard:(h + 1) * S_shard, :],
            "key": key[b],
            "value": value[b],
            "Wq": Wq, "Wk": Wk, "Wv": Wv,
        })
    res = run_bass_kernel_spmd(nc, in_maps, core_ids=list(range(N_CORES)))
    y = np.empty((B, S_FULL, D_MODEL), dtype=np.float32)
    for i in range(N_CORES):
        b, h = i // 2, i % 2
        y[b, h * S_shard:(h + 1) * S_shard, :] = res.results[i]["out"]
    return y
